# revision 1
# baseline (speedup 1.0000x reference)
"""Multi-head attention (B=2, S=2048, D=1024, H=16) on 8 Trainium2 cores.

Sharding: head-parallel. Core c handles head pair {2c, 2c+1} (GP=128 proj
dims) for BOTH batches -> per-core attention load is balanced across cores
regardless of the per-batch valid_seq_lens.

Masking: reference masks scores to -1e6 => exp == 0.0 exactly in fp32, so
key tiles entirely beyond valid_len contribute nothing to numerator or
denominator and are skipped outright (the kernel is compiled per
ceil(valid_len/128) pair, cached). The partial last tile is handled by
zeroing masked V rows on host (numerator) and a 0/1 mask column appended
as a 65th V column whose attn-output row accumulates the softmax
denominator (masked positions excluded) for free.

Per-core math (bf16 matmuls, fp32 PSUM accum):
  QT[b] = (Wq/8).T @ xq[b].T        [128, 2048]
  KT[b] = Wk.T @ xk[b].T            [128, SKb]   (SKb = 128*ceil(vl_b/128))
  V[b]  = xv[b] @ Wv                [SKb, 128]   (masked rows zeroed)
  per head h (rows h*64..h*64+63 of QT/KT):
    S^T = K_h @ Q_h^T per 128-key tile, p = exp(S^T) (scores O(1), no max)
    [O_h^T; denom] += [V_h | mask].T @ p
  OT[b] = O^T / denom               (reciprocal + gpsimd broadcast)
  out[b] partial = O @ Wo_rows      [2048, 1024]  (host sums 8 partials)

Inputs are host-prepacked to [P, block, chunk, seq] so every block DMA is
one contiguous 2-8KB line per partition (128 descriptors, not 1024).
Output/aux DMAs go through the sync engine (HWDGE) - the Q7 SWDGE path
serializes descriptor generation.
"""

import math
import numpy as np
from contextlib import ExitStack, nullcontext

B, S, D, H = 2, 2048, 1024, 16
DH = 64
GP = 128  # per-core projection width: 2 heads
P = 128
QB = 512
NQB = S // QB
NKC = D // P  # 8 contraction chunks over D
DH1 = DH + 1  # V columns + mask column

_BUILD_CACHE = {}


def _kt_blocks(nktb):
    """Split nktb key tiles into blocks of <=4 tiles (<=512 columns)."""
    out = []
    kt0 = 0
    while kt0 < nktb:
        ktn = min(4, nktb - kt0)
        out.append((kt0, ktn))
        kt0 += ktn
    return out


def _build(nkt=(13, 9), reps=1, loop_n=1):
    key = (nkt, reps, loop_n)
    if key in _BUILD_CACHE:
        return _BUILD_CACHE[key]
    import concourse.bass as bass
    import concourse.tile as tile
    from concourse import bacc, mybir

    f32 = mybir.dt.float32
    bf16 = mybir.dt.bfloat16
    SK = [nkt[0] * P, nkt[1] * P]

    nc = bacc.Bacc("TRN2", target_bir_lowering=False, debug=False, num_devices=8)

    xq = [nc.dram_tensor(f"xq{b}_p", [P, NQB, NKC, QB], bf16,
                         kind="ExternalInput").ap() for b in range(B)]
    xk = [nc.dram_tensor(f"xk{b}_p", [P, nkt[b], NKC, P], bf16,
                         kind="ExternalInput").ap() for b in range(B)]
    xv = [nc.dram_tensor(f"xv{b}_p", [P, nkt[b], NKC, P], bf16,
                         kind="ExternalInput").ap() for b in range(B)]
    wq = nc.dram_tensor("wq", [D, GP], bf16, kind="ExternalInput").ap()
    wk = nc.dram_tensor("wk", [D, GP], bf16, kind="ExternalInput").ap()
    wv = nc.dram_tensor("wv", [D, GP], bf16, kind="ExternalInput").ap()
    wo = nc.dram_tensor("wo", [GP, D], bf16, kind="ExternalInput").ap()
    maskd = [nc.dram_tensor(f"mask{b}_t", [P, nkt[b]], bf16, kind="ExternalInput").ap()
             for b in range(B)]
    outp = [nc.dram_tensor(f"out{b}", [S, D], bf16, kind="ExternalOutput").ap()
            for b in range(B)]

    with tile.TileContext(nc) as tc:
        with ExitStack() as ctx:
            wpool = ctx.enter_context(tc.tile_pool(name="weights", bufs=1))
            xpool = ctx.enter_context(tc.tile_pool(name="xstream", bufs=3))
            qkpool = ctx.enter_context(tc.tile_pool(name="qk", bufs=1))
            vpool = ctx.enter_context(tc.tile_pool(name="v", bufs=1))
            otpool = ctx.enter_context(tc.tile_pool(name="ot", bufs=1))
            ppool = ctx.enter_context(tc.tile_pool(name="p", bufs=6))
            rcpool = ctx.enter_context(tc.tile_pool(name="rc", bufs=4))
            bcpool = ctx.enter_context(tc.tile_pool(name="bc", bufs=4))
            opool = ctx.enter_context(tc.tile_pool(name="oev", bufs=4))
            # PSUM budget (8 banks): 2 score slots x 2 banks + 2 attn
            # accumulators x 1 bank + 2 proj/Wo slots x 1 bank.
            ps_s = ctx.enter_context(tc.tile_pool(name="ps_s", bufs=2, space="PSUM"))
            ps_o = ctx.enter_context(tc.tile_pool(name="ps_o", bufs=2, space="PSUM"))
            ps_p = ctx.enter_context(tc.tile_pool(name="ps_p", bufs=2, space="PSUM"))

            # ---- resident weights (loaded once, outside the bench loop) ----
            wq_s = wpool.tile([P, NKC, GP], bf16, tag="wq")
            wk_s = wpool.tile([P, NKC, GP], bf16, tag="wk")
            wv_s = wpool.tile([P, NKC, GP], bf16, tag="wv")
            wo_s = wpool.tile([P, D], bf16, tag="wo")
            mask_s = [wpool.tile([P, nkt[b]], bf16, tag=f"mask{b}", name=f"mask{b}")
                      for b in range(B)]
            nc.sync.dma_start(wk_s[:], wk.rearrange("(c p) m -> p c m", p=P))
            nc.sync.dma_start(wv_s[:], wv.rearrange("(c p) m -> p c m", p=P))
            nc.sync.dma_start(wq_s[:], wq.rearrange("(c p) m -> p c m", p=P))
            nc.sync.dma_start(wo_s[:], wo[:])
            for b in range(B):
                nc.sync.dma_start(mask_s[b][:], maskd[b][:])

            with (tc.For_i(0, loop_n, 1) if loop_n > 1 else nullcontext()):
              for _ in range(reps):
                # ---- per-iteration residents ----
                QT = [qkpool.tile([P, S], bf16, tag=f"qt{b}", name=f"qt{b}")
                      for b in range(B)]
                KT = [qkpool.tile([P, SK[b]], bf16, tag=f"kt{b}", name=f"kt{b}")
                      for b in range(B)]
                OT = [otpool.tile([P, S], bf16, tag=f"ot{b}", name=f"ot{b}")
                      for b in range(B)]
                V_sb = [vpool.tile([P, nkt[b], 2, DH1], bf16, tag=f"vsb{b}",
                                   name=f"vsb{b}")
                        for b in range(B)]

                fillers = []

                def pump(n=1):
                    for _ in range(n):
                        if fillers:
                            fillers.pop(0)()

                # ---- projection work units ----
                def q_proj_block(b, qb):
                    xt = xpool.tile([P, NKC, QB], bf16, tag="xs", name="xt")
                    nc.sync.dma_start(xt[:], xq[b][:, qb])
                    ps = ps_p.tile([P, QB], f32, tag="acc", name="ps")
                    for c in range(NKC):
                        nc.tensor.matmul(ps[:], lhsT=wq_s[:, c, :], rhs=xt[:, c, :],
                                         start=(c == 0), stop=(c == NKC - 1))
                    nc.vector.tensor_copy(QT[b][:, qb * QB:(qb + 1) * QB], ps[:])

                def k_proj_block(b, kt0, ktn):
                    ncol = ktn * P
                    xt = xpool.tile([P, ktn, NKC, P], bf16, tag="xs", name="xt")
                    nc.sync.dma_start(xt[:], xk[b][:, kt0:kt0 + ktn])
                    ps = ps_p.tile([P, QB], f32, tag="acc", name="ps")
                    for c in range(NKC):
                        nc.tensor.matmul(ps[:, 0:ncol], lhsT=wk_s[:, c, :],
                                         rhs=xt[:, :, c, :],
                                         start=(c == 0), stop=(c == NKC - 1))
                    nc.vector.tensor_copy(
                        KT[b][:, kt0 * P: kt0 * P + ncol], ps[:, 0:ncol])

                def v_proj_block(b, kt0, ktn):
                    xt = xpool.tile([P, ktn, NKC, P], bf16, tag="xs", name="xt")
                    nc.sync.dma_start(xt[:], xv[b][:, kt0:kt0 + ktn])
                    ps = ps_p.tile([P, QB], f32, tag="acc", name="ps")
                    for i in range(ktn):
                        for c in range(NKC):
                            nc.tensor.matmul(ps[:, i * P:(i + 1) * P],
                                             lhsT=xt[:, i, c, :],
                                             rhs=wv_s[:, c, :],
                                             start=(c == 0), stop=(c == NKC - 1))
                    nc.vector.tensor_copy(
                        V_sb[b][:, kt0:kt0 + ktn, :, 0:DH],
                        ps[:, 0:ktn * P].rearrange("p (s h d) -> p s h d",
                                                   s=ktn, h=2))
                    for i in range(ktn):
                        for hp in range(2):
                            nc.gpsimd.tensor_copy(
                                V_sb[b][:, kt0 + i, hp, DH:DH1],
                                mask_s[b][:, kt0 + i:kt0 + i + 1])

                def wo_st(b, st):
                    ssl = slice(st * P, (st + 1) * P)
                    osb = opool.tile([P, 2, QB], bf16, tag="osb", name="osb")
                    for nh in range(2):
                        ps = ps_p.tile([P, QB], f32, tag="acc", name="wops")
                        nc.tensor.matmul(ps[:], lhsT=OT[b][:, ssl],
                                         rhs=wo_s[:, nh * QB:(nh + 1) * QB],
                                         start=True, stop=True)
                        nc.vector.tensor_copy(osb[:, nh, :], ps[:])
                    nc.sync.dma_start(outp[b][ssl, :], osb[:])

                # ---- attention for one (batch, q-block) ----
                def attn_qb(b, qb):
                    qsl = slice(qb * QB, (qb + 1) * QB)
                    ot_ps = [ps_o.tile([DH1, QB], f32, tag="acc", name=f"otps{i}")
                             for i in range(2)]
                    nktb = nkt[b]
                    for kt in range(nktb):
                        s_ps = ps_s.tile([P, 2, QB], f32, tag="s", name="sps")
                        for hp in range(2):
                            hsl = slice(hp * DH, (hp + 1) * DH)
                            nc.tensor.matmul(
                                s_ps[:, hp, :],
                                lhsT=KT[b][hsl, kt * P:(kt + 1) * P],
                                rhs=QT[b][hsl, qsl],
                                start=True, stop=True)
                        pt = ppool.tile([P, 2, QB], bf16, tag="p", name="pt")
                        nc.scalar.activation(
                            pt[:], s_ps[:], bass.mybir.ActivationFunctionType.Exp)
                        for hp in range(2):
                            nc.tensor.matmul(
                                ot_ps[hp][:],
                                lhsT=V_sb[b][:, kt, hp, :],
                                rhs=pt[:, hp, :],
                                start=(kt == 0), stop=(kt == nktb - 1))
                        pump(1)
                    # Drain PSUM fast (unnormalized O + denominator) so the
                    # accumulator banks free up for the next q-block; the
                    # normalization chain then runs off the critical path.
                    ou, den = [], []
                    for hp in range(2):
                        o_t = opool.tile([DH, QB], bf16, tag="ou", name="o_t")
                        nc.vector.tensor_copy(o_t[:], ot_ps[hp][0:DH, :])
                        d_t = rcpool.tile([DH1, QB], f32, tag="den", name="d_t")
                        nc.vector.tensor_copy(d_t[DH:DH1, :],
                                              ot_ps[hp][DH:DH1, :])
                        ou.append(o_t)
                        den.append(d_t)
                    for hp in range(2):
                        rc = rcpool.tile([DH1, QB], f32, tag="rc", name="rc")
                        with nc.allow_low_precision(reason="softmax recip"):
                            nc.vector.reciprocal(
                                rc[DH:DH1, :], den[hp][DH:DH1, :])
                        rcb = rcpool.tile([DH1, QB], bf16, tag="rcb", name="rcb")
                        nc.gpsimd.tensor_copy(rcb[DH:DH1, :], rc[DH:DH1, :])
                        rc0 = rcpool.tile([1, QB], bf16, tag="rc0", name="rc0")
                        nc.sync.dma_start(rc0[0:1, :], rcb[DH:DH1, :])
                        bc = bcpool.tile([P, QB], bf16, tag="bc", name="bc")
                        nc.gpsimd.partition_broadcast(bc[:], rc0[0:1, :])
                        with nc.allow_low_precision(reason="bf16 attn out"):
                            if hp == 0:
                                nc.vector.tensor_mul(
                                    OT[b][0:DH, qsl], ou[0][:], bc[0:DH, :])
                            else:
                                oev = opool.tile([DH, QB], bf16, tag="oev",
                                                 name="oev")
                                nc.vector.tensor_mul(
                                    oev[:], ou[1][:], bc[0:DH, :])
                                nc.sync.dma_start(OT[b][DH:P, qsl], oev[:])

                # ---- emission: startup block, then weave fillers ----
                def kv_units(b):
                    us = []
                    for kt0, ktn in _kt_blocks(nkt[b]):
                        us.append(lambda bb=b, k0=kt0, kn=ktn:
                                  k_proj_block(bb, k0, kn))
                        us.append(lambda bb=b, k0=kt0, kn=ktn:
                                  v_proj_block(bb, k0, kn))
                    return us

                def wo_units(b, s0, s1):
                    return [lambda bb=b, s=st: wo_st(bb, s) for st in range(s0, s1)]

                def q_unit(b, qb):
                    return [lambda bb=b, q=qb: q_proj_block(bb, q)]

                start = kv_units(0)
                start[0]()  # K b0 blk0
                start[1]()  # V b0 blk0
                q_proj_block(0, 0)
                fillers.extend(start[2:] + q_unit(0, 1))

                u1 = kv_units(1)
                half = (len(u1) + 1) // 2
                sched = [
                    ((0, 1), u1[:half] + q_unit(0, 2) + wo_units(0, 0, 4)),
                    ((0, 2), u1[half:] + q_unit(0, 3) + wo_units(0, 4, 8)),
                    ((0, 3), q_unit(1, 0) + wo_units(0, 8, 12)),
                    ((1, 0), q_unit(1, 1) + wo_units(0, 12, 16)),
                    ((1, 1), q_unit(1, 2) + wo_units(1, 0, 4)),
                    ((1, 2), q_unit(1, 3) + wo_units(1, 4, 8)),
                    ((1, 3), wo_units(1, 8, 12)),
                ]

                attn_qb(0, 0)
                for (b, qb), units in sched:
                    while fillers:  # drain leftovers: order correctness
                        fillers.pop(0)()
                    fillers.extend(units)
                    attn_qb(b, qb)
                while fillers:
                    fillers.pop(0)()
                for u in wo_units(1, 12, 16):
                    u()

    nc.compile()
    _BUILD_CACHE[key] = nc
    return nc


def _prep_inputs(queries, keys, values, Wq, Wk, Wv, Wo, valid_seq_lens):
    import ml_dtypes
    bf16 = ml_dtypes.bfloat16

    qn = np.asarray(queries, dtype=np.float32)
    kn = np.asarray(keys, dtype=np.float32)
    vn = np.asarray(values, dtype=np.float32)
    wqn = (np.asarray(Wq, dtype=np.float32) * np.float32(1.0 / np.sqrt(DH))).astype(bf16)
    wkn = np.asarray(Wk, dtype=np.float32).astype(bf16)
    wvn = np.asarray(Wv, dtype=np.float32).astype(bf16)
    won = np.asarray(Wo, dtype=np.float32).astype(bf16)
    vl = np.asarray(valid_seq_lens).astype(np.int64)
    nkt = tuple(int(math.ceil(int(vl[b]) / P)) for b in range(B))

    shared = {}
    for b in range(B):
        sk = nkt[b] * P
        vmask = (np.arange(S) < vl[b]).astype(np.float32)
        vb = vn[b] * vmask[:, None]
        # packed [P, block, chunk, seq]: one contiguous line per partition
        xq_t = qn[b].T.astype(bf16)  # [D, S]
        shared[f"xq{b}_p"] = np.ascontiguousarray(
            xq_t.reshape(NKC, P, NQB, QB).transpose(1, 2, 0, 3))
        xk_t = kn[b].T[:, :sk].astype(bf16)
        shared[f"xk{b}_p"] = np.ascontiguousarray(
            xk_t.reshape(NKC, P, nkt[b], P).transpose(1, 2, 0, 3))
        xv_t = vb.T[:, :sk].astype(bf16)
        shared[f"xv{b}_p"] = np.ascontiguousarray(
            xv_t.reshape(NKC, P, nkt[b], P).transpose(1, 2, 0, 3))
        shared[f"mask{b}_t"] = np.ascontiguousarray(
            vmask[:sk].reshape(nkt[b], P).T).astype(bf16)

    in_maps = []
    for core in range(8):
        gsl = slice(core * GP, (core + 1) * GP)
        m = dict(shared)
        m["wq"] = np.ascontiguousarray(wqn[:, gsl])
        m["wk"] = np.ascontiguousarray(wkn[:, gsl])
        m["wv"] = np.ascontiguousarray(wvn[:, gsl])
        m["wo"] = np.ascontiguousarray(won[gsl, :])
        in_maps.append(m)
    return in_maps, nkt


def kernel(queries, keys, values, Wq, Wk, Wv, Wo, valid_seq_lens):
    from concourse.bass_utils import run_bass_kernel_spmd

    in_maps, nkt = _prep_inputs(
        queries, keys, values, Wq, Wk, Wv, Wo, valid_seq_lens)
    nc = _build(nkt)
    res = run_bass_kernel_spmd(nc, in_maps, list(range(8)))
    out = np.zeros((B, S, D), dtype=np.float32)
    for core in range(8):
        for b in range(B):
            out[b] += res.results[core][f"out{b}"].astype(np.float32)
    return out



# revision 20
# speedup vs baseline: 1.0872x; 1.0872x over previous
"""Multi-head attention (B=2, S=2048, D=1024, H=16) on 8 Trainium2 cores.

Sharding: head-parallel. Core c handles head pair {2c, 2c+1} (GP=128 proj
dims) for BOTH batches -> per-core attention load is balanced across cores
regardless of the per-batch valid_seq_lens.

Masking: reference masks scores to -1e6 => exp == 0.0 exactly in fp32, so
key tiles entirely beyond valid_len contribute nothing to numerator or
denominator and are skipped outright (the kernel is compiled per
ceil(valid_len/128) pair, cached). The partial last tile is handled by
zeroing masked V rows on host (numerator) and a 0/1 mask column appended
as a 65th V column whose attn-output row accumulates the softmax
denominator (masked positions excluded) for free.

Per-core math (bf16 matmuls, fp32 PSUM accum):
  QT[b] = (Wq/8).T @ xq[b].T        [128, 2048]
  KT[b] = Wk.T @ xk[b].T            [128, SKb]   (SKb = 128*ceil(vl_b/128))
  V[b]  = xv[b] @ Wv                [SKb, 128]   (masked rows zeroed)
  per head h (rows h*64..h*64+63 of QT/KT):
    S^T = K_h @ Q_h^T per 128-key tile, p = exp(S^T) (scores O(1), no max)
    [O_h^T; denom] += [V_h | mask].T @ p
  OT[b] = O^T / denom               (reciprocal + gpsimd broadcast)
  out[b] partial = O @ Wo_rows      [2048, 1024]  (host sums 8 partials)

Inputs are host-prepacked to [P, block, chunk, seq] so every block DMA is
one contiguous 2-8KB line per partition (128 descriptors, not 1024).
Output/aux DMAs go through the sync engine (HWDGE) - the Q7 SWDGE path
serializes descriptor generation.
"""

import math
import numpy as np
from contextlib import ExitStack, nullcontext

B, S, D, H = 2, 2048, 1024, 16
DH = 64
GP = 128  # per-core projection width: 2 heads
P = 128
QB = 512
NQB = S // QB
NKC = D // P  # 8 contraction chunks over D
DH1 = DH + 1  # V columns + mask column

_BUILD_CACHE = {}


def _kt_blocks(nktb):
    """Split nktb key tiles into blocks of <=4 tiles (<=512 columns)."""
    out = []
    kt0 = 0
    while kt0 < nktb:
        ktn = min(4, nktb - kt0)
        out.append((kt0, ktn))
        kt0 += ktn
    return out


def _build(nkt=(13, 9), reps=1, loop_n=1, variant="base"):
    key = (nkt, reps, loop_n, variant)
    if key in _BUILD_CACHE:
        return _BUILD_CACHE[key]
    va = set(variant.split("+")) - {"base"}
    import concourse.bass as bass
    import concourse.tile as tile
    from concourse import bacc, mybir

    f32 = mybir.dt.float32
    bf16 = mybir.dt.bfloat16
    SK = [nkt[0] * P, nkt[1] * P]

    nc = bacc.Bacc("TRN2", target_bir_lowering=False, debug=False, num_devices=8)

    xq = [nc.dram_tensor(f"xq{b}_p", [P, NQB, NKC, QB], bf16,
                         kind="ExternalInput").ap() for b in range(B)]
    xk = [nc.dram_tensor(f"xk{b}_p", [P, nkt[b], NKC, P], bf16,
                         kind="ExternalInput").ap() for b in range(B)]
    xv = [nc.dram_tensor(f"xv{b}_p", [P, nkt[b], NKC, P], bf16,
                         kind="ExternalInput").ap() for b in range(B)]
    wq = nc.dram_tensor("wq", [D, GP], bf16, kind="ExternalInput").ap()
    wk = nc.dram_tensor("wk", [D, GP], bf16, kind="ExternalInput").ap()
    wv = nc.dram_tensor("wv", [D, GP], bf16, kind="ExternalInput").ap()
    wo = nc.dram_tensor("wo", [GP, D], bf16, kind="ExternalInput").ap()
    maskd = [nc.dram_tensor(f"mask{b}_t", [P, nkt[b], 2], bf16,
                            kind="ExternalInput").ap()
             for b in range(B)]
    outp = [nc.dram_tensor(f"out{b}", [S, D], bf16, kind="ExternalOutput").ap()
            for b in range(B)]

    with tile.TileContext(nc) as tc:
        with ExitStack() as ctx:
            wpool = ctx.enter_context(tc.tile_pool(name="weights", bufs=1))
            xpool = ctx.enter_context(tc.tile_pool(name="xstream", bufs=3))
            qkpool = ctx.enter_context(tc.tile_pool(name="qk", bufs=1))
            vpool = ctx.enter_context(tc.tile_pool(name="v", bufs=1))
            otpool = ctx.enter_context(tc.tile_pool(name="ot", bufs=1))
            ppool = ctx.enter_context(tc.tile_pool(name="p", bufs=6))
            rcpool = ctx.enter_context(tc.tile_pool(name="rc", bufs=4))
            bcpool = ctx.enter_context(tc.tile_pool(name="bc", bufs=4))
            opool = ctx.enter_context(tc.tile_pool(name="oev", bufs=4))
            # PSUM budget (8 banks): 2 score slots x 2 banks + 2 attn
            # accumulators x 1 bank + 2 proj/Wo slots x 1 bank.
            ps_s = ctx.enter_context(tc.tile_pool(name="ps_s", bufs=2, space="PSUM"))
            ps_o = ctx.enter_context(tc.tile_pool(name="ps_o", bufs=2, space="PSUM"))
            ps_p = ctx.enter_context(tc.tile_pool(name="ps_p", bufs=2, space="PSUM"))

            # ---- resident weights (loaded once, outside the bench loop) ----
            wq_s = wpool.tile([P, NKC, GP], bf16, tag="wq")
            wk_s = wpool.tile([P, NKC, GP], bf16, tag="wk")
            wv_s = wpool.tile([P, NKC, GP], bf16, tag="wv")
            wo_s = wpool.tile([P, D], bf16, tag="wo")
            mask_s = [wpool.tile([P, nkt[b], 2], bf16, tag=f"mask{b}",
                                 name=f"mask{b}")
                      for b in range(B)]
            nc.sync.dma_start(wk_s[:], wk.rearrange("(c p) m -> p c m", p=P))
            nc.sync.dma_start(wv_s[:], wv.rearrange("(c p) m -> p c m", p=P))
            nc.sync.dma_start(wq_s[:], wq.rearrange("(c p) m -> p c m", p=P))
            nc.sync.dma_start(wo_s[:], wo[:])
            for b in range(B):
                nc.sync.dma_start(mask_s[b][:], maskd[b][:])

            with (tc.For_i(0, loop_n, 1) if loop_n > 1 else nullcontext()):
              for _ in range(reps):
                # ---- per-iteration residents ----
                QT = [qkpool.tile([P, S], bf16, tag=f"qt{b}", name=f"qt{b}")
                      for b in range(B)]
                KT = [qkpool.tile([P, SK[b]], bf16, tag=f"kt{b}", name=f"kt{b}")
                      for b in range(B)]
                OT = [otpool.tile([P, S], bf16, tag=f"ot{b}", name=f"ot{b}")
                      for b in range(B)]
                V_sb = [vpool.tile([P, nkt[b], 2, DH1], bf16, tag=f"vsb{b}",
                                   name=f"vsb{b}")
                        for b in range(B)]

                fillers = []

                def pump(n=1):
                    for _ in range(n):
                        if fillers:
                            fillers.pop(0)()

                # ---- projection work units ----
                def q_proj_block(b, qb):
                    xt = xpool.tile([P, NKC, QB], bf16, tag="xs", name="xt")
                    if "nodma" not in va:
                        nc.sync.dma_start(xt[:], xq[b][:, qb])
                    else:
                        nc.gpsimd.memset(xt[:, 0, 0:1], 0.0)
                    if "noproj" in va:
                        nc.vector.memset(QT[b][:, qb * QB:qb * QB + 1], 0.0)
                        return
                    ps = ps_p.tile([P, QB], f32, tag="acc", name="ps")
                    for c in range(NKC):
                        nc.tensor.matmul(ps[:], lhsT=wq_s[:, c, :], rhs=xt[:, c, :],
                                         start=(c == 0), stop=(c == NKC - 1))
                    nc.vector.tensor_copy(QT[b][:, qb * QB:(qb + 1) * QB], ps[:])

                def k_proj_block(b, kt0, ktn):
                    ncol = ktn * P
                    xt = xpool.tile([P, ktn, NKC, P], bf16, tag="xs", name="xt")
                    if "nodma" not in va:
                        nc.sync.dma_start(xt[:], xk[b][:, kt0:kt0 + ktn])
                    else:
                        nc.gpsimd.memset(xt[:, 0, 0, 0:1], 0.0)
                    if "noproj" in va:
                        nc.vector.memset(KT[b][:, kt0 * P:kt0 * P + 1], 0.0)
                        return
                    ps = ps_p.tile([P, QB], f32, tag="acc", name="ps")
                    for c in range(NKC):
                        nc.tensor.matmul(ps[:, 0:ncol], lhsT=wk_s[:, c, :],
                                         rhs=xt[:, :, c, :],
                                         start=(c == 0), stop=(c == NKC - 1))
                    nc.vector.tensor_copy(
                        KT[b][:, kt0 * P: kt0 * P + ncol], ps[:, 0:ncol])

                def v_proj_block(b, kt0, ktn):
                    xt = xpool.tile([P, ktn, NKC, P], bf16, tag="xs", name="xt")
                    if "nodma" not in va:
                        nc.sync.dma_start(xt[:], xv[b][:, kt0:kt0 + ktn])
                    else:
                        nc.gpsimd.memset(xt[:, 0, 0, 0:1], 0.0)
                    if "noproj" in va:
                        nc.vector.memset(V_sb[b][:, kt0, 0, 0:1], 0.0)
                        return
                    ps = ps_p.tile([P, QB], f32, tag="acc", name="ps")
                    for i in range(ktn):
                        for c in range(NKC):
                            nc.tensor.matmul(ps[:, i * P:(i + 1) * P],
                                             lhsT=xt[:, i, c, :],
                                             rhs=wv_s[:, c, :],
                                             start=(c == 0), stop=(c == NKC - 1))
                    nc.vector.tensor_copy(
                        V_sb[b][:, kt0:kt0 + ktn, :, 0:DH],
                        ps[:, 0:ktn * P].rearrange("p (s h d) -> p s h d",
                                                   s=ktn, h=2))
                    nc.vector.tensor_copy(
                        V_sb[b][:, kt0:kt0 + ktn, :, DH:DH1],
                        mask_s[b][:, kt0:kt0 + ktn, :])

                def wo_st(b, st):
                    if "nowo" in va:
                        return
                    ssl = slice(st * P, (st + 1) * P)
                    osb = opool.tile([P, 2, QB], bf16, tag="osb", name="osb")
                    for nh in range(2):
                        ps = ps_p.tile([P, QB], f32, tag="acc", name="wops")
                        nc.tensor.matmul(ps[:], lhsT=OT[b][:, ssl],
                                         rhs=wo_s[:, nh * QB:(nh + 1) * QB],
                                         start=True, stop=True)
                        # split the PSUM drain between DVE and ACT so neither
                        # engine owns the full 2x2048x1024 output cast
                        if nh == 0:
                            nc.vector.tensor_copy(osb[:, nh, :], ps[:])
                        else:
                            nc.scalar.copy(osb[:, nh, :], ps[:])
                    nc.sync.dma_start(outp[b][ssl, :], osb[:])

                # ---- attention for one (batch, q-block) ----
                def attn_qb(b, qb):
                    qsl = slice(qb * QB, (qb + 1) * QB)
                    ot_ps = [ps_o.tile([DH1, QB], f32, tag="acc", name=f"otps{i}")
                             for i in range(2)]
                    nktb = nkt[b]
                    for kt in range(nktb):
                        s_ps = ps_s.tile([P, 2, QB], f32, tag="s", name="sps")
                        if "noscore" not in va:
                            for hp in range(2):
                                hsl = slice(hp * DH, (hp + 1) * DH)
                                nc.tensor.matmul(
                                    s_ps[:, hp, :],
                                    lhsT=KT[b][hsl, kt * P:(kt + 1) * P],
                                    rhs=QT[b][hsl, qsl],
                                    start=True, stop=True)
                        pt = ppool.tile([P, 2, QB], bf16, tag="p", name="pt")
                        if "noscore" in va:
                            nc.vector.memset(s_ps[:, :, 0:1], 0.0)
                        if "noexp" in va:
                            nc.vector.memset(pt[:, :, 0:1], 0.0)
                        elif "exphalf" in va:
                            nc.scalar.activation(
                                pt[:, :, 0:128], s_ps[:, :, 0:128],
                                bass.mybir.ActivationFunctionType.Exp)
                        else:
                            nc.scalar.activation(
                                pt[:], s_ps[:],
                                bass.mybir.ActivationFunctionType.Exp)
                        if "noattnv" not in va:
                            for hp in range(2):
                                nc.tensor.matmul(
                                    ot_ps[hp][:],
                                    lhsT=V_sb[b][:, kt, hp, :],
                                    rhs=pt[:, hp, :],
                                    start=(kt == 0), stop=(kt == nktb - 1))
                        pump(1)
                    if "noattnv" in va or "nonorm" in va:
                        nc.vector.memset(OT[b][:, qsl][:, 0:1], 0.0)
                        return
                    # Short normalization chain straight off PSUM: recip of
                    # the denominator row (partition 64) -> broadcast ->
                    # multiply the unnormalized O while it still sits in the
                    # accumulator bank. head1's result lands on partitions
                    # 0-63 and is moved up by a DVE stream_shuffle (the APs'
                    # base partitions carry the +64 shift).
                    for hp in range(2):
                        if "dtcopy" in va:
                            d_t = rcpool.tile([DH1, QB], bf16, tag="den",
                                              name="d_t")
                            nc.vector.tensor_copy(d_t[DH:DH1, :],
                                                  ot_ps[hp][DH:DH1, :])
                            den_src = d_t[DH:DH1, :]
                        else:
                            den_src = ot_ps[hp][DH:DH1, :]
                        rc = rcpool.tile([DH1, QB], bf16, tag="rc", name="rc")
                        with nc.allow_low_precision(reason="softmax recip"):
                            nc.vector.reciprocal(rc[DH:DH1, :], den_src)
                        # partition_broadcast requires a partition-0 source on
                        # HW (base-64 APs silently read partition 0); hop the
                        # rc row down via the otherwise-idle ACT HWDGE queue.
                        bc = bcpool.tile([P, QB], bf16, tag="bc", name="bc")
                        rc0 = rcpool.tile([1, QB], bf16, tag="rc0", name="rc0")
                        if "rc0sp" in va:
                            nc.sync.dma_start(rc0[0:1, :], rc[DH:DH1, :])
                        else:
                            nc.scalar.dma_start(rc0[0:1, :], rc[DH:DH1, :])
                        nc.gpsimd.partition_broadcast(bc[:], rc0[0:1, :])
                        with nc.allow_low_precision(reason="bf16 attn out"):
                            if hp == 0:
                                nc.vector.tensor_mul(
                                    OT[b][0:DH, qsl], ot_ps[0][0:DH, :],
                                    bc[0:DH, :])
                            else:
                                oev = opool.tile([DH, QB], bf16, tag="oev",
                                                 name="oev")
                                nc.vector.tensor_mul(
                                    oev[:], ot_ps[1][0:DH, :], bc[0:DH, :])
                                if "oevdma" in va:
                                    nc.sync.dma_start(OT[b][DH:P, qsl], oev[:])
                                else:
                                    nc.vector.stream_shuffle(
                                        OT[b][DH:P, qsl], oev[:],
                                        mask=list(range(32)))

                # ---- emission: startup block, then weave fillers ----
                def kv_units(b):
                    us = []
                    for kt0, ktn in _kt_blocks(nkt[b]):
                        us.append(lambda bb=b, k0=kt0, kn=ktn:
                                  k_proj_block(bb, k0, kn))
                        us.append(lambda bb=b, k0=kt0, kn=ktn:
                                  v_proj_block(bb, k0, kn))
                    return us

                def wo_units(b, s0, s1):
                    return [lambda bb=b, s=st: wo_st(bb, s) for st in range(s0, s1)]

                def q_unit(b, qb):
                    return [lambda bb=b, q=qb: q_proj_block(bb, q)]

                start = kv_units(0)
                start[0]()  # K b0 blk0
                start[1]()  # V b0 blk0
                q_proj_block(0, 0)
                fillers.extend(start[2:] + q_unit(0, 1))

                u1 = kv_units(1)
                half = (len(u1) + 1) // 2
                sched = [
                    ((0, 1), u1[:half] + q_unit(0, 2)),
                    ((0, 2), u1[half:] + q_unit(0, 3) + wo_units(0, 0, 4)),
                    ((0, 3), q_unit(1, 0) + wo_units(0, 4, 8)),
                    ((1, 0), q_unit(1, 1) + wo_units(0, 8, 12)),
                    ((1, 1), q_unit(1, 2) + wo_units(0, 12, 16)),
                    ((1, 2), q_unit(1, 3) + wo_units(1, 0, 4)),
                    ((1, 3), wo_units(1, 4, 8)),
                ]

                attn_qb(0, 0)
                for (b, qb), units in sched:
                    while fillers:  # drain leftovers: order correctness
                        fillers.pop(0)()
                    fillers.extend(units)
                    attn_qb(b, qb)
                while fillers:
                    fillers.pop(0)()
                for u in wo_units(1, 8, 16):
                    u()

    nc.compile()
    _BUILD_CACHE[key] = nc
    return nc


def _prep_inputs(queries, keys, values, Wq, Wk, Wv, Wo, valid_seq_lens):
    import ml_dtypes
    bf16 = ml_dtypes.bfloat16

    qn = np.asarray(queries, dtype=np.float32)
    kn = np.asarray(keys, dtype=np.float32)
    vn = np.asarray(values, dtype=np.float32)
    wqn = (np.asarray(Wq, dtype=np.float32) * np.float32(1.0 / np.sqrt(DH))).astype(bf16)
    wkn = np.asarray(Wk, dtype=np.float32).astype(bf16)
    wvn = np.asarray(Wv, dtype=np.float32).astype(bf16)
    won = np.asarray(Wo, dtype=np.float32).astype(bf16)
    vl = np.asarray(valid_seq_lens).astype(np.int64)
    nkt = tuple(int(math.ceil(int(vl[b]) / P)) for b in range(B))

    shared = {}
    for b in range(B):
        sk = nkt[b] * P
        vmask = (np.arange(S) < vl[b]).astype(np.float32)
        vb = vn[b] * vmask[:, None]
        # packed [P, block, chunk, seq]: one contiguous line per partition
        xq_t = qn[b].T.astype(bf16)  # [D, S]
        shared[f"xq{b}_p"] = np.ascontiguousarray(
            xq_t.reshape(NKC, P, NQB, QB).transpose(1, 2, 0, 3))
        xk_t = kn[b].T[:, :sk].astype(bf16)
        shared[f"xk{b}_p"] = np.ascontiguousarray(
            xk_t.reshape(NKC, P, nkt[b], P).transpose(1, 2, 0, 3))
        xv_t = vb.T[:, :sk].astype(bf16)
        shared[f"xv{b}_p"] = np.ascontiguousarray(
            xv_t.reshape(NKC, P, nkt[b], P).transpose(1, 2, 0, 3))
        mt = vmask[:sk].reshape(nkt[b], P).T.astype(bf16)  # [P, nkt]
        shared[f"mask{b}_t"] = np.ascontiguousarray(
            np.repeat(mt[:, :, None], 2, axis=2))

    in_maps = []
    for core in range(8):
        gsl = slice(core * GP, (core + 1) * GP)
        m = dict(shared)
        m["wq"] = np.ascontiguousarray(wqn[:, gsl])
        m["wk"] = np.ascontiguousarray(wkn[:, gsl])
        m["wv"] = np.ascontiguousarray(wvn[:, gsl])
        m["wo"] = np.ascontiguousarray(won[gsl, :])
        in_maps.append(m)
    return in_maps, nkt


def kernel(queries, keys, values, Wq, Wk, Wv, Wo, valid_seq_lens):
    from concourse.bass_utils import run_bass_kernel_spmd

    in_maps, nkt = _prep_inputs(
        queries, keys, values, Wq, Wk, Wv, Wo, valid_seq_lens)
    nc = _build(nkt)
    res = run_bass_kernel_spmd(nc, in_maps, list(range(8)))
    out = np.zeros((B, S, D), dtype=np.float32)
    for core in range(8):
        for b in range(B):
            out[b] += res.results[core][f"out{b}"].astype(np.float32)
    return out



# revision 21
# speedup vs baseline: 1.0994x; 1.0113x over previous
"""Multi-head attention (B=2, S=2048, D=1024, H=16) on 8 Trainium2 cores.

Sharding: head-parallel. Core c handles head pair {2c, 2c+1} (GP=128 proj
dims) for BOTH batches -> per-core attention load is balanced across cores
regardless of the per-batch valid_seq_lens.

Masking: reference masks scores to -1e6 => exp == 0.0 exactly in fp32, so
key tiles entirely beyond valid_len contribute nothing to numerator or
denominator and are skipped outright (the kernel is compiled per
ceil(valid_len/128) pair, cached). The partial last tile is handled by
zeroing masked V rows on host (numerator) and a 0/1 mask column appended
as a 65th V column whose attn-output row accumulates the softmax
denominator (masked positions excluded) for free.

Per-core math (bf16 matmuls, fp32 PSUM accum):
  QT[b] = (Wq/8).T @ xq[b].T        [128, 2048]
  KT[b] = Wk.T @ xk[b].T            [128, SKb]   (SKb = 128*ceil(vl_b/128))
  V[b]  = xv[b] @ Wv                [SKb, 128]   (masked rows zeroed)
  per head h (rows h*64..h*64+63 of QT/KT):
    S^T = K_h @ Q_h^T per 128-key tile, p = exp(S^T) (scores O(1), no max)
    [O_h^T; denom] += [V_h | mask].T @ p
  OT[b] = O^T / denom               (reciprocal + gpsimd broadcast)
  out[b] partial = O @ Wo_rows      [2048, 1024]  (host sums 8 partials)

Inputs are host-prepacked to [P, block, chunk, seq] so every block DMA is
one contiguous 2-8KB line per partition (128 descriptors, not 1024).
Output/aux DMAs go through the sync engine (HWDGE) - the Q7 SWDGE path
serializes descriptor generation.
"""

import math
import numpy as np
from contextlib import ExitStack, nullcontext

B, S, D, H = 2, 2048, 1024, 16
DH = 64
GP = 128  # per-core projection width: 2 heads
P = 128
QB = 512
NQB = S // QB
NKC = D // P  # 8 contraction chunks over D
DH1 = DH + 1  # V columns + mask column

_BUILD_CACHE = {}


def _kt_blocks(nktb):
    """Split nktb key tiles into blocks of <=4 tiles (<=512 columns)."""
    out = []
    kt0 = 0
    while kt0 < nktb:
        ktn = min(4, nktb - kt0)
        out.append((kt0, ktn))
        kt0 += ktn
    return out


def _build(nkt=(13, 9), reps=1, loop_n=1, variant="base"):
    key = (nkt, reps, loop_n, variant)
    if key in _BUILD_CACHE:
        return _BUILD_CACHE[key]
    va = set(variant.split("+")) - {"base"}
    import concourse.bass as bass
    import concourse.tile as tile
    from concourse import bacc, mybir

    f32 = mybir.dt.float32
    bf16 = mybir.dt.bfloat16
    SK = [nkt[0] * P, nkt[1] * P]

    nc = bacc.Bacc("TRN2", target_bir_lowering=False, debug=False, num_devices=8)

    xq = [nc.dram_tensor(f"xq{b}_p", [P, NQB, NKC, QB], bf16,
                         kind="ExternalInput").ap() for b in range(B)]
    xk = [nc.dram_tensor(f"xk{b}_p", [P, nkt[b], NKC, P], bf16,
                         kind="ExternalInput").ap() for b in range(B)]
    xv = [nc.dram_tensor(f"xv{b}_p", [P, nkt[b], NKC, P], bf16,
                         kind="ExternalInput").ap() for b in range(B)]
    wq = nc.dram_tensor("wq", [D, GP], bf16, kind="ExternalInput").ap()
    wk = nc.dram_tensor("wk", [D, GP], bf16, kind="ExternalInput").ap()
    wv = nc.dram_tensor("wv", [D, GP], bf16, kind="ExternalInput").ap()
    wo = nc.dram_tensor("wo", [GP, D], bf16, kind="ExternalInput").ap()
    maskd = [nc.dram_tensor(f"mask{b}_t", [P, nkt[b], 2], bf16,
                            kind="ExternalInput").ap()
             for b in range(B)]
    outp = [nc.dram_tensor(f"out{b}", [S, D], bf16, kind="ExternalOutput").ap()
            for b in range(B)]

    with tile.TileContext(nc) as tc:
        with ExitStack() as ctx:
            wpool = ctx.enter_context(tc.tile_pool(name="weights", bufs=1))
            xpool = ctx.enter_context(tc.tile_pool(name="xstream", bufs=3))
            qkpool = ctx.enter_context(tc.tile_pool(name="qk", bufs=1))
            vpool = ctx.enter_context(tc.tile_pool(name="v", bufs=1))
            otpool = ctx.enter_context(tc.tile_pool(name="ot", bufs=1))
            ppool = ctx.enter_context(tc.tile_pool(name="p", bufs=6))
            rcpool = ctx.enter_context(tc.tile_pool(name="rc", bufs=4))
            bcpool = ctx.enter_context(tc.tile_pool(name="bc", bufs=4))
            opool = ctx.enter_context(tc.tile_pool(name="oev", bufs=4))
            # PSUM budget (8 banks): 2 score slots x 2 banks + 2 attn
            # accumulators x 1 bank + 2 proj/Wo slots x 1 bank.
            ps_s = ctx.enter_context(tc.tile_pool(name="ps_s", bufs=2, space="PSUM"))
            ps_o = ctx.enter_context(tc.tile_pool(name="ps_o", bufs=2, space="PSUM"))
            ps_p = ctx.enter_context(tc.tile_pool(name="ps_p", bufs=2, space="PSUM"))

            # ---- resident weights (loaded once, outside the bench loop) ----
            wq_s = wpool.tile([P, NKC, GP], bf16, tag="wq")
            wk_s = wpool.tile([P, NKC, GP], bf16, tag="wk")
            wv_s = wpool.tile([P, NKC, GP], bf16, tag="wv")
            wo_s = wpool.tile([P, D], bf16, tag="wo")
            mask_s = [wpool.tile([P, nkt[b], 2], bf16, tag=f"mask{b}",
                                 name=f"mask{b}")
                      for b in range(B)]
            nc.sync.dma_start(wk_s[:], wk.rearrange("(c p) m -> p c m", p=P))
            nc.sync.dma_start(wv_s[:], wv.rearrange("(c p) m -> p c m", p=P))
            nc.sync.dma_start(wq_s[:], wq.rearrange("(c p) m -> p c m", p=P))
            nc.sync.dma_start(wo_s[:], wo[:])
            for b in range(B):
                nc.sync.dma_start(mask_s[b][:], maskd[b][:])

            with (tc.For_i(0, loop_n, 1) if loop_n > 1 else nullcontext()):
              for _ in range(reps):
                # ---- per-iteration residents ----
                QT = [qkpool.tile([P, S], bf16, tag=f"qt{b}", name=f"qt{b}")
                      for b in range(B)]
                KT = [qkpool.tile([P, SK[b]], bf16, tag=f"kt{b}", name=f"kt{b}")
                      for b in range(B)]
                OT = [otpool.tile([P, S], bf16, tag=f"ot{b}", name=f"ot{b}")
                      for b in range(B)]
                V_sb = [vpool.tile([P, nkt[b], 2, DH1], bf16, tag=f"vsb{b}",
                                   name=f"vsb{b}")
                        for b in range(B)]

                fillers = []

                def pump(n=1):
                    for _ in range(n):
                        if fillers:
                            fillers.pop(0)()

                # ---- projection work units ----
                def q_proj_block(b, qb):
                    xt = xpool.tile([P, NKC, QB], bf16, tag="xs", name="xt")
                    if "nodma" not in va:
                        nc.sync.dma_start(xt[:], xq[b][:, qb])
                    else:
                        nc.gpsimd.memset(xt[:, 0, 0:1], 0.0)
                    if "noproj" in va:
                        nc.vector.memset(QT[b][:, qb * QB:qb * QB + 1], 0.0)
                        return
                    ps = ps_p.tile([P, QB], f32, tag="acc", name="ps")
                    for c in range(NKC):
                        nc.tensor.matmul(ps[:], lhsT=wq_s[:, c, :], rhs=xt[:, c, :],
                                         start=(c == 0), stop=(c == NKC - 1))
                    nc.vector.tensor_copy(QT[b][:, qb * QB:(qb + 1) * QB], ps[:])

                def k_proj_block(b, kt0, ktn):
                    ncol = ktn * P
                    xt = xpool.tile([P, ktn, NKC, P], bf16, tag="xs", name="xt")
                    if "nodma" not in va:
                        nc.sync.dma_start(xt[:], xk[b][:, kt0:kt0 + ktn])
                    else:
                        nc.gpsimd.memset(xt[:, 0, 0, 0:1], 0.0)
                    if "noproj" in va:
                        nc.vector.memset(KT[b][:, kt0 * P:kt0 * P + 1], 0.0)
                        return
                    ps = ps_p.tile([P, QB], f32, tag="acc", name="ps")
                    for c in range(NKC):
                        nc.tensor.matmul(ps[:, 0:ncol], lhsT=wk_s[:, c, :],
                                         rhs=xt[:, :, c, :],
                                         start=(c == 0), stop=(c == NKC - 1))
                    nc.vector.tensor_copy(
                        KT[b][:, kt0 * P: kt0 * P + ncol], ps[:, 0:ncol])

                def v_proj_block(b, kt0, ktn):
                    xt = xpool.tile([P, ktn, NKC, P], bf16, tag="xs", name="xt")
                    if "nodma" not in va:
                        nc.sync.dma_start(xt[:], xv[b][:, kt0:kt0 + ktn])
                    else:
                        nc.gpsimd.memset(xt[:, 0, 0, 0:1], 0.0)
                    if "noproj" in va:
                        nc.vector.memset(V_sb[b][:, kt0, 0, 0:1], 0.0)
                        return
                    ps = ps_p.tile([P, QB], f32, tag="acc", name="ps")
                    for i in range(ktn):
                        for c in range(NKC):
                            nc.tensor.matmul(ps[:, i * P:(i + 1) * P],
                                             lhsT=xt[:, i, c, :],
                                             rhs=wv_s[:, c, :],
                                             start=(c == 0), stop=(c == NKC - 1))
                    nc.vector.tensor_copy(
                        V_sb[b][:, kt0:kt0 + ktn, :, 0:DH],
                        ps[:, 0:ktn * P].rearrange("p (s h d) -> p s h d",
                                                   s=ktn, h=2))
                    nc.vector.tensor_copy(
                        V_sb[b][:, kt0:kt0 + ktn, :, DH:DH1],
                        mask_s[b][:, kt0:kt0 + ktn, :])

                def wo_st(b, st):
                    if "nowo" in va:
                        return
                    ssl = slice(st * P, (st + 1) * P)
                    osb = opool.tile([P, 2, QB], bf16, tag="osb", name="osb")
                    for nh in range(2):
                        ps = ps_p.tile([P, QB], f32, tag="acc", name="wops")
                        nc.tensor.matmul(ps[:], lhsT=OT[b][:, ssl],
                                         rhs=wo_s[:, nh * QB:(nh + 1) * QB],
                                         start=True, stop=True)
                        # split the PSUM drain between DVE and ACT so neither
                        # engine owns the full 2x2048x1024 output cast
                        if nh == 0:
                            nc.vector.tensor_copy(osb[:, nh, :], ps[:])
                        else:
                            nc.scalar.copy(osb[:, nh, :], ps[:])
                    nc.sync.dma_start(outp[b][ssl, :], osb[:])

                # ---- attention for one (batch, q-block) ----
                def attn_qb(b, qb):
                    qsl = slice(qb * QB, (qb + 1) * QB)
                    ot_ps = [ps_o.tile([DH1, QB], f32, tag="acc", name=f"otps{i}")
                             for i in range(2)]
                    nktb = nkt[b]
                    for kt in range(nktb):
                        s_ps = ps_s.tile([P, 2, QB], f32, tag="s", name="sps")
                        if "noscore" not in va:
                            for hp in range(2):
                                hsl = slice(hp * DH, (hp + 1) * DH)
                                nc.tensor.matmul(
                                    s_ps[:, hp, :],
                                    lhsT=KT[b][hsl, kt * P:(kt + 1) * P],
                                    rhs=QT[b][hsl, qsl],
                                    start=True, stop=True)
                        pt = ppool.tile([P, 2, QB], bf16, tag="p", name="pt")
                        if "noscore" in va:
                            nc.vector.memset(s_ps[:, :, 0:1], 0.0)
                        if "noexp" in va:
                            nc.vector.memset(pt[:, :, 0:1], 0.0)
                        elif "exphalf" in va:
                            nc.scalar.activation(
                                pt[:, :, 0:128], s_ps[:, :, 0:128],
                                bass.mybir.ActivationFunctionType.Exp)
                        else:
                            nc.scalar.activation(
                                pt[:], s_ps[:],
                                bass.mybir.ActivationFunctionType.Exp)
                        if "noattnv" not in va:
                            for hp in range(2):
                                nc.tensor.matmul(
                                    ot_ps[hp][:],
                                    lhsT=V_sb[b][:, kt, hp, :],
                                    rhs=pt[:, hp, :],
                                    start=(kt == 0), stop=(kt == nktb - 1))
                        pump(1)
                    if "noattnv" in va or "nonorm" in va:
                        nc.vector.memset(OT[b][:, qsl][:, 0:1], 0.0)
                        return
                    # Short normalization chain straight off PSUM: recip of
                    # the denominator row (partition 64) -> broadcast ->
                    # multiply the unnormalized O while it still sits in the
                    # accumulator bank. head1's result lands on partitions
                    # 0-63 and is moved up by a DVE stream_shuffle (the APs'
                    # base partitions carry the +64 shift).
                    # One fast bf16 drain per head releases the PSUM
                    # accumulator bank immediately (the next q-block's attnV
                    # waits on it); the slow recip/broadcast chain then runs
                    # entirely off SBUF and only gates wo, scheduled 2+ slots
                    # later.
                    ou = []
                    for hp in range(2):
                        o_t = opool.tile([DH1, QB], bf16, tag="ou", name="o_t")
                        nc.vector.tensor_copy(o_t[:], ot_ps[hp][:])
                        ou.append(o_t)
                    for hp in range(2):
                        rc = rcpool.tile([DH1, QB], bf16, tag="rc", name="rc")
                        with nc.allow_low_precision(reason="softmax recip"):
                            nc.vector.reciprocal(rc[DH:DH1, :],
                                                 ou[hp][DH:DH1, :])
                        # partition_broadcast requires a partition-0 source on
                        # HW (base-64 APs silently read partition 0); hop the
                        # rc row down via the otherwise-idle ACT HWDGE queue.
                        bc = bcpool.tile([P, QB], bf16, tag="bc", name="bc")
                        rc0 = rcpool.tile([1, QB], bf16, tag="rc0", name="rc0")
                        if "rc0sp" in va:
                            nc.sync.dma_start(rc0[0:1, :], rc[DH:DH1, :])
                        else:
                            nc.scalar.dma_start(rc0[0:1, :], rc[DH:DH1, :])
                        nc.gpsimd.partition_broadcast(bc[:], rc0[0:1, :])
                        with nc.allow_low_precision(reason="bf16 attn out"):
                            if hp == 0:
                                nc.vector.tensor_mul(
                                    OT[b][0:DH, qsl], ou[0][0:DH, :],
                                    bc[0:DH, :])
                            else:
                                oev = opool.tile([DH, QB], bf16, tag="oev",
                                                 name="oev")
                                nc.vector.tensor_mul(
                                    oev[:], ou[1][0:DH, :], bc[0:DH, :])
                                if "oevdma" in va:
                                    nc.sync.dma_start(OT[b][DH:P, qsl], oev[:])
                                else:
                                    nc.vector.stream_shuffle(
                                        OT[b][DH:P, qsl], oev[:],
                                        mask=list(range(32)))

                # ---- emission: startup block, then weave fillers ----
                def kv_units(b):
                    us = []
                    for kt0, ktn in _kt_blocks(nkt[b]):
                        us.append(lambda bb=b, k0=kt0, kn=ktn:
                                  k_proj_block(bb, k0, kn))
                        us.append(lambda bb=b, k0=kt0, kn=ktn:
                                  v_proj_block(bb, k0, kn))
                    return us

                def wo_units(b, s0, s1):
                    return [lambda bb=b, s=st: wo_st(bb, s) for st in range(s0, s1)]

                def q_unit(b, qb):
                    return [lambda bb=b, q=qb: q_proj_block(bb, q)]

                start = kv_units(0)
                start[0]()  # K b0 blk0
                start[1]()  # V b0 blk0
                q_proj_block(0, 0)
                fillers.extend(start[2:] + q_unit(0, 1))

                u1 = kv_units(1)
                half = (len(u1) + 1) // 2
                sched = [
                    ((0, 1), u1[:half] + q_unit(0, 2)),
                    ((0, 2), u1[half:] + q_unit(0, 3) + wo_units(0, 0, 4)),
                    ((0, 3), q_unit(1, 0) + wo_units(0, 4, 8)),
                    ((1, 0), q_unit(1, 1) + wo_units(0, 8, 12)),
                    ((1, 1), q_unit(1, 2) + wo_units(0, 12, 16)),
                    ((1, 2), q_unit(1, 3) + wo_units(1, 0, 4)),
                    ((1, 3), wo_units(1, 4, 8)),
                ]

                attn_qb(0, 0)
                for (b, qb), units in sched:
                    while fillers:  # drain leftovers: order correctness
                        fillers.pop(0)()
                    fillers.extend(units)
                    attn_qb(b, qb)
                while fillers:
                    fillers.pop(0)()
                for u in wo_units(1, 8, 16):
                    u()

    nc.compile()
    _BUILD_CACHE[key] = nc
    return nc


def _prep_inputs(queries, keys, values, Wq, Wk, Wv, Wo, valid_seq_lens):
    import ml_dtypes
    bf16 = ml_dtypes.bfloat16

    qn = np.asarray(queries, dtype=np.float32)
    kn = np.asarray(keys, dtype=np.float32)
    vn = np.asarray(values, dtype=np.float32)
    wqn = (np.asarray(Wq, dtype=np.float32) * np.float32(1.0 / np.sqrt(DH))).astype(bf16)
    wkn = np.asarray(Wk, dtype=np.float32).astype(bf16)
    wvn = np.asarray(Wv, dtype=np.float32).astype(bf16)
    won = np.asarray(Wo, dtype=np.float32).astype(bf16)
    vl = np.asarray(valid_seq_lens).astype(np.int64)
    nkt = tuple(int(math.ceil(int(vl[b]) / P)) for b in range(B))

    shared = {}
    for b in range(B):
        sk = nkt[b] * P
        vmask = (np.arange(S) < vl[b]).astype(np.float32)
        vb = vn[b] * vmask[:, None]
        # packed [P, block, chunk, seq]: one contiguous line per partition
        xq_t = qn[b].T.astype(bf16)  # [D, S]
        shared[f"xq{b}_p"] = np.ascontiguousarray(
            xq_t.reshape(NKC, P, NQB, QB).transpose(1, 2, 0, 3))
        xk_t = kn[b].T[:, :sk].astype(bf16)
        shared[f"xk{b}_p"] = np.ascontiguousarray(
            xk_t.reshape(NKC, P, nkt[b], P).transpose(1, 2, 0, 3))
        xv_t = vb.T[:, :sk].astype(bf16)
        shared[f"xv{b}_p"] = np.ascontiguousarray(
            xv_t.reshape(NKC, P, nkt[b], P).transpose(1, 2, 0, 3))
        mt = vmask[:sk].reshape(nkt[b], P).T.astype(bf16)  # [P, nkt]
        shared[f"mask{b}_t"] = np.ascontiguousarray(
            np.repeat(mt[:, :, None], 2, axis=2))

    in_maps = []
    for core in range(8):
        gsl = slice(core * GP, (core + 1) * GP)
        m = dict(shared)
        m["wq"] = np.ascontiguousarray(wqn[:, gsl])
        m["wk"] = np.ascontiguousarray(wkn[:, gsl])
        m["wv"] = np.ascontiguousarray(wvn[:, gsl])
        m["wo"] = np.ascontiguousarray(won[gsl, :])
        in_maps.append(m)
    return in_maps, nkt


def kernel(queries, keys, values, Wq, Wk, Wv, Wo, valid_seq_lens):
    from concourse.bass_utils import run_bass_kernel_spmd

    in_maps, nkt = _prep_inputs(
        queries, keys, values, Wq, Wk, Wv, Wo, valid_seq_lens)
    nc = _build(nkt)
    res = run_bass_kernel_spmd(nc, in_maps, list(range(8)))
    out = np.zeros((B, S, D), dtype=np.float32)
    for core in range(8):
        for b in range(B):
            out[b] += res.results[core][f"out{b}"].astype(np.float32)
    return out



# revision 22
# speedup vs baseline: 1.1202x; 1.0189x over previous
"""Multi-head attention (B=2, S=2048, D=1024, H=16) on 8 Trainium2 cores.

Sharding: head-parallel. Core c handles head pair {2c, 2c+1} (GP=128 proj
dims) for BOTH batches -> per-core attention load is balanced across cores
regardless of the per-batch valid_seq_lens.

Masking: reference masks scores to -1e6 => exp == 0.0 exactly in fp32, so
key tiles entirely beyond valid_len contribute nothing to numerator or
denominator and are skipped outright (the kernel is compiled per
ceil(valid_len/128) pair, cached). The partial last tile is handled by
zeroing masked V rows on host (numerator) and a 0/1 mask column appended
as a 65th V column whose attn-output row accumulates the softmax
denominator (masked positions excluded) for free.

Per-core math (bf16 matmuls, fp32 PSUM accum):
  QT[b] = (Wq/8).T @ xq[b].T        [128, 2048]
  KT[b] = Wk.T @ xk[b].T            [128, SKb]   (SKb = 128*ceil(vl_b/128))
  V[b]  = xv[b] @ Wv                [SKb, 128]   (masked rows zeroed)
  per head h (rows h*64..h*64+63 of QT/KT):
    S^T = K_h @ Q_h^T per 128-key tile, p = exp(S^T) (scores O(1), no max)
    [O_h^T; denom] += [V_h | mask].T @ p
  OT[b] = O^T / denom               (reciprocal + gpsimd broadcast)
  out[b] partial = O @ Wo_rows      [2048, 1024]  (host sums 8 partials)

Inputs are host-prepacked to [P, block, chunk, seq] so every block DMA is
one contiguous 2-8KB line per partition (128 descriptors, not 1024).
Output/aux DMAs go through the sync engine (HWDGE) - the Q7 SWDGE path
serializes descriptor generation.
"""

import math
import numpy as np
from contextlib import ExitStack, nullcontext

B, S, D, H = 2, 2048, 1024, 16
DH = 64
GP = 128  # per-core projection width: 2 heads
P = 128
QB = 512
NQB = S // QB
NKC = D // P  # 8 contraction chunks over D
DH1 = DH + 1  # V columns + mask column

_BUILD_CACHE = {}


def _kt_blocks(nktb):
    """Split nktb key tiles into blocks of <=4 tiles (<=512 columns)."""
    out = []
    kt0 = 0
    while kt0 < nktb:
        ktn = min(4, nktb - kt0)
        out.append((kt0, ktn))
        kt0 += ktn
    return out


def _build(nkt=(13, 9), reps=1, loop_n=1, variant="base"):
    key = (nkt, reps, loop_n, variant)
    if key in _BUILD_CACHE:
        return _BUILD_CACHE[key]
    va = set(variant.split("+")) - {"base"}
    import concourse.bass as bass
    import concourse.tile as tile
    from concourse import bacc, mybir

    f32 = mybir.dt.float32
    bf16 = mybir.dt.bfloat16
    SK = [nkt[0] * P, nkt[1] * P]

    nc = bacc.Bacc("TRN2", target_bir_lowering=False, debug=False, num_devices=8)

    xq = [nc.dram_tensor(f"xq{b}_p", [P, NQB, NKC, QB], bf16,
                         kind="ExternalInput").ap() for b in range(B)]
    xk = [nc.dram_tensor(f"xk{b}_p", [P, nkt[b], NKC, P], bf16,
                         kind="ExternalInput").ap() for b in range(B)]
    xv = [nc.dram_tensor(f"xv{b}_p", [P, nkt[b], NKC, P], bf16,
                         kind="ExternalInput").ap() for b in range(B)]
    wq = nc.dram_tensor("wq", [D, GP], bf16, kind="ExternalInput").ap()
    wk = nc.dram_tensor("wk", [D, GP], bf16, kind="ExternalInput").ap()
    wv = nc.dram_tensor("wv", [D, GP], bf16, kind="ExternalInput").ap()
    wo = nc.dram_tensor("wo", [GP, D], bf16, kind="ExternalInput").ap()
    maskd = [nc.dram_tensor(f"mask{b}_t", [P, nkt[b], 2], bf16,
                            kind="ExternalInput").ap()
             for b in range(B)]
    outp = [nc.dram_tensor(f"out{b}", [S, D], bf16, kind="ExternalOutput").ap()
            for b in range(B)]

    with tile.TileContext(nc) as tc:
        with ExitStack() as ctx:
            wpool = ctx.enter_context(tc.tile_pool(name="weights", bufs=1))
            xpool = ctx.enter_context(tc.tile_pool(name="xstream", bufs=3))
            qkpool = ctx.enter_context(tc.tile_pool(name="qk", bufs=1))
            vpool = ctx.enter_context(tc.tile_pool(name="v", bufs=1))
            otpool = ctx.enter_context(tc.tile_pool(name="ot", bufs=1))
            ppool = ctx.enter_context(tc.tile_pool(name="p", bufs=6))
            rcpool = ctx.enter_context(tc.tile_pool(name="rc", bufs=4))
            bcpool = ctx.enter_context(tc.tile_pool(name="bc", bufs=4))
            opool = ctx.enter_context(tc.tile_pool(name="oev", bufs=4))
            # PSUM budget (8 banks): 2 score slots x 2 banks + 2 attn
            # accumulators x 1 bank + 2 proj/Wo slots x 1 bank.
            ps_s = ctx.enter_context(tc.tile_pool(name="ps_s", bufs=2, space="PSUM"))
            ps_o = ctx.enter_context(tc.tile_pool(name="ps_o", bufs=2, space="PSUM"))
            ps_p = ctx.enter_context(tc.tile_pool(name="ps_p", bufs=2, space="PSUM"))

            # ---- resident weights (loaded once, outside the bench loop) ----
            wq_s = wpool.tile([P, NKC, GP], bf16, tag="wq")
            wk_s = wpool.tile([P, NKC, GP], bf16, tag="wk")
            wv_s = wpool.tile([P, NKC, GP], bf16, tag="wv")
            wo_s = wpool.tile([P, D], bf16, tag="wo")
            mask_s = [wpool.tile([P, nkt[b], 2], bf16, tag=f"mask{b}",
                                 name=f"mask{b}")
                      for b in range(B)]
            nc.sync.dma_start(wk_s[:], wk.rearrange("(c p) m -> p c m", p=P))
            nc.sync.dma_start(wv_s[:], wv.rearrange("(c p) m -> p c m", p=P))
            nc.sync.dma_start(wq_s[:], wq.rearrange("(c p) m -> p c m", p=P))
            nc.sync.dma_start(wo_s[:], wo[:])
            for b in range(B):
                nc.sync.dma_start(mask_s[b][:], maskd[b][:])

            with (tc.For_i(0, loop_n, 1) if loop_n > 1 else nullcontext()):
              for _ in range(reps):
                # ---- per-iteration residents ----
                QT = [qkpool.tile([P, S], bf16, tag=f"qt{b}", name=f"qt{b}")
                      for b in range(B)]
                KT = [qkpool.tile([P, SK[b]], bf16, tag=f"kt{b}", name=f"kt{b}")
                      for b in range(B)]
                OT = [otpool.tile([P, S], bf16, tag=f"ot{b}", name=f"ot{b}")
                      for b in range(B)]
                V_sb = [vpool.tile([P, nkt[b], 2, DH1], bf16, tag=f"vsb{b}",
                                   name=f"vsb{b}")
                        for b in range(B)]

                fillers = []

                def pump(n=1):
                    for _ in range(n):
                        if fillers:
                            fillers.pop(0)()

                # ---- projection work units ----
                def q_proj_block(b, qb):
                    xt = xpool.tile([P, NKC, QB], bf16, tag="xs", name="xt")
                    if "nodma" not in va:
                        nc.sync.dma_start(xt[:], xq[b][:, qb])
                    else:
                        nc.gpsimd.memset(xt[:, 0, 0:1], 0.0)
                    if "noproj" in va:
                        nc.vector.memset(QT[b][:, qb * QB:qb * QB + 1], 0.0)
                        return
                    ps = ps_p.tile([P, QB], f32, tag="acc", name="ps")
                    for c in range(NKC):
                        nc.tensor.matmul(ps[:], lhsT=wq_s[:, c, :], rhs=xt[:, c, :],
                                         start=(c == 0), stop=(c == NKC - 1))
                    nc.vector.tensor_copy(QT[b][:, qb * QB:(qb + 1) * QB], ps[:])

                def k_proj_block(b, kt0, ktn):
                    ncol = ktn * P
                    xt = xpool.tile([P, ktn, NKC, P], bf16, tag="xs", name="xt")
                    if "nodma" not in va:
                        nc.sync.dma_start(xt[:], xk[b][:, kt0:kt0 + ktn])
                    else:
                        nc.gpsimd.memset(xt[:, 0, 0, 0:1], 0.0)
                    if "noproj" in va:
                        nc.vector.memset(KT[b][:, kt0 * P:kt0 * P + 1], 0.0)
                        return
                    ps = ps_p.tile([P, QB], f32, tag="acc", name="ps")
                    for c in range(NKC):
                        nc.tensor.matmul(ps[:, 0:ncol], lhsT=wk_s[:, c, :],
                                         rhs=xt[:, :, c, :],
                                         start=(c == 0), stop=(c == NKC - 1))
                    nc.vector.tensor_copy(
                        KT[b][:, kt0 * P: kt0 * P + ncol], ps[:, 0:ncol])

                def v_proj_block(b, kt0, ktn):
                    xt = xpool.tile([P, ktn, NKC, P], bf16, tag="xs", name="xt")
                    if "nodma" not in va:
                        nc.sync.dma_start(xt[:], xv[b][:, kt0:kt0 + ktn])
                    else:
                        nc.gpsimd.memset(xt[:, 0, 0, 0:1], 0.0)
                    if "noproj" in va:
                        nc.vector.memset(V_sb[b][:, kt0, 0, 0:1], 0.0)
                        return
                    ps = ps_p.tile([P, QB], f32, tag="acc", name="ps")
                    for i in range(ktn):
                        for c in range(NKC):
                            nc.tensor.matmul(ps[:, i * P:(i + 1) * P],
                                             lhsT=xt[:, i, c, :],
                                             rhs=wv_s[:, c, :],
                                             start=(c == 0), stop=(c == NKC - 1))
                    nc.vector.tensor_copy(
                        V_sb[b][:, kt0:kt0 + ktn, :, 0:DH],
                        ps[:, 0:ktn * P].rearrange("p (s h d) -> p s h d",
                                                   s=ktn, h=2))
                    nc.vector.tensor_copy(
                        V_sb[b][:, kt0:kt0 + ktn, :, DH:DH1],
                        mask_s[b][:, kt0:kt0 + ktn, :])

                def wo_st(b, st):
                    if "nowo" in va:
                        return
                    ssl = slice(st * P, (st + 1) * P)
                    osb = opool.tile([P, 2, QB], bf16, tag="osb", name="osb")
                    for nh in range(2):
                        ps = ps_p.tile([P, QB], f32, tag="acc", name="wops")
                        nc.tensor.matmul(ps[:], lhsT=OT[b][:, ssl],
                                         rhs=wo_s[:, nh * QB:(nh + 1) * QB],
                                         start=True, stop=True)
                        # split the PSUM drain between DVE and ACT so neither
                        # engine owns the full 2x2048x1024 output cast
                        if nh == 0:
                            nc.vector.tensor_copy(osb[:, nh, :], ps[:])
                        else:
                            nc.scalar.copy(osb[:, nh, :], ps[:])
                    nc.sync.dma_start(outp[b][ssl, :], osb[:])

                # ---- attention for one (batch, q-block) ----
                def attn_qb(b, qb):
                    qsl = slice(qb * QB, (qb + 1) * QB)
                    ot_ps = [ps_o.tile([DH1, QB], f32, tag="acc", name=f"otps{i}")
                             for i in range(2)]
                    nktb = nkt[b]
                    for kt in range(nktb):
                        s_ps = ps_s.tile([P, 2, QB], f32, tag="s", name="sps")
                        if "noscore" not in va:
                            for hp in range(2):
                                hsl = slice(hp * DH, (hp + 1) * DH)
                                nc.tensor.matmul(
                                    s_ps[:, hp, :],
                                    lhsT=KT[b][hsl, kt * P:(kt + 1) * P],
                                    rhs=QT[b][hsl, qsl],
                                    start=True, stop=True)
                        pt = ppool.tile([P, 2, QB], bf16, tag="p", name="pt")
                        if "noscore" in va:
                            nc.vector.memset(s_ps[:, :, 0:1], 0.0)
                        if "noexp" in va:
                            nc.vector.memset(pt[:, :, 0:1], 0.0)
                        elif "exphalf" in va:
                            nc.scalar.activation(
                                pt[:, :, 0:128], s_ps[:, :, 0:128],
                                bass.mybir.ActivationFunctionType.Exp)
                        else:
                            nc.scalar.activation(
                                pt[:], s_ps[:],
                                bass.mybir.ActivationFunctionType.Exp)
                        if "noattnv" not in va:
                            for hp in range(2):
                                nc.tensor.matmul(
                                    ot_ps[hp][:],
                                    lhsT=V_sb[b][:, kt, hp, :],
                                    rhs=pt[:, hp, :],
                                    start=(kt == 0), stop=(kt == nktb - 1))
                        pump(1)
                    if "noattnv" in va or "nonorm" in va:
                        nc.vector.memset(OT[b][:, qsl][:, 0:1], 0.0)
                        return
                    # Short normalization chain straight off PSUM: recip of
                    # the denominator row (partition 64) -> broadcast ->
                    # multiply the unnormalized O while it still sits in the
                    # accumulator bank. head1's result lands on partitions
                    # 0-63 and is moved up by a DVE stream_shuffle (the APs'
                    # base partitions carry the +64 shift).
                    # Two fast bf16 drains release the PSUM accumulator banks
                    # immediately (the next q-block's attnV waits on them);
                    # the slow recip/broadcast chain then runs entirely off
                    # SBUF, merged across both heads, and only gates wo,
                    # scheduled 2+ slots later.
                    ou = opool.tile([DH1, 2, QB], bf16, tag="ou", name="ou")
                    for hp in range(2):
                        nc.vector.tensor_copy(ou[:, hp, :], ot_ps[hp][:])
                    rc = rcpool.tile([DH1, 2, QB], bf16, tag="rc", name="rc")
                    with nc.allow_low_precision(reason="softmax recip"):
                        nc.vector.reciprocal(rc[DH:DH1, :, :],
                                             ou[DH:DH1, :, :])
                    # partition_broadcast requires a partition-0 source on
                    # HW (base-64 APs silently read partition 0); hop the
                    # rc row down via the otherwise-idle ACT HWDGE queue.
                    bc = bcpool.tile([P, 2, QB], bf16, tag="bc", name="bc")
                    rc0 = rcpool.tile([1, 2, QB], bf16, tag="rc0", name="rc0")
                    if "rc0sp" in va:
                        nc.sync.dma_start(rc0[0:1, :, :], rc[DH:DH1, :, :])
                    else:
                        nc.scalar.dma_start(rc0[0:1, :, :], rc[DH:DH1, :, :])
                    nc.gpsimd.partition_broadcast(bc[:], rc0[0:1, :, :])
                    with nc.allow_low_precision(reason="bf16 attn out"):
                        nc.vector.tensor_mul(
                            OT[b][0:DH, qsl], ou[0:DH, 0, :], bc[0:DH, 0, :])
                        oev = opool.tile([DH, QB], bf16, tag="oev", name="oev")
                        nc.vector.tensor_mul(
                            oev[:], ou[0:DH, 1, :], bc[0:DH, 1, :])
                        if "oevdma" in va:
                            nc.sync.dma_start(OT[b][DH:P, qsl], oev[:])
                        else:
                            nc.vector.stream_shuffle(
                                OT[b][DH:P, qsl], oev[:],
                                mask=list(range(32)))

                # ---- emission: startup block, then weave fillers ----
                def kv_units(b):
                    us = []
                    for kt0, ktn in _kt_blocks(nkt[b]):
                        us.append(lambda bb=b, k0=kt0, kn=ktn:
                                  k_proj_block(bb, k0, kn))
                        us.append(lambda bb=b, k0=kt0, kn=ktn:
                                  v_proj_block(bb, k0, kn))
                    return us

                def wo_units(b, s0, s1):
                    return [lambda bb=b, s=st: wo_st(bb, s) for st in range(s0, s1)]

                def q_unit(b, qb):
                    return [lambda bb=b, q=qb: q_proj_block(bb, q)]

                start = kv_units(0)
                start[0]()  # K b0 blk0
                start[1]()  # V b0 blk0
                q_proj_block(0, 0)
                fillers.extend(start[2:] + q_unit(0, 1))

                u1 = kv_units(1)
                half = (len(u1) + 1) // 2
                sched = [
                    ((0, 1), u1[:half] + q_unit(0, 2)),
                    ((0, 2), u1[half:] + q_unit(0, 3) + wo_units(0, 0, 4)),
                    ((0, 3), q_unit(1, 0) + wo_units(0, 4, 8)),
                    ((1, 0), q_unit(1, 1) + wo_units(0, 8, 12)),
                    ((1, 1), q_unit(1, 2) + wo_units(0, 12, 16)),
                    ((1, 2), q_unit(1, 3) + wo_units(1, 0, 4)),
                    ((1, 3), wo_units(1, 4, 8)),
                ]

                attn_qb(0, 0)
                for (b, qb), units in sched:
                    while fillers:  # drain leftovers: order correctness
                        fillers.pop(0)()
                    fillers.extend(units)
                    attn_qb(b, qb)
                while fillers:
                    fillers.pop(0)()
                for u in wo_units(1, 8, 16):
                    u()

    nc.compile()
    _BUILD_CACHE[key] = nc
    return nc


def _prep_inputs(queries, keys, values, Wq, Wk, Wv, Wo, valid_seq_lens):
    import ml_dtypes
    bf16 = ml_dtypes.bfloat16

    qn = np.asarray(queries, dtype=np.float32)
    kn = np.asarray(keys, dtype=np.float32)
    vn = np.asarray(values, dtype=np.float32)
    wqn = (np.asarray(Wq, dtype=np.float32) * np.float32(1.0 / np.sqrt(DH))).astype(bf16)
    wkn = np.asarray(Wk, dtype=np.float32).astype(bf16)
    wvn = np.asarray(Wv, dtype=np.float32).astype(bf16)
    won = np.asarray(Wo, dtype=np.float32).astype(bf16)
    vl = np.asarray(valid_seq_lens).astype(np.int64)
    nkt = tuple(int(math.ceil(int(vl[b]) / P)) for b in range(B))

    shared = {}
    for b in range(B):
        sk = nkt[b] * P
        vmask = (np.arange(S) < vl[b]).astype(np.float32)
        vb = vn[b] * vmask[:, None]
        # packed [P, block, chunk, seq]: one contiguous line per partition
        xq_t = qn[b].T.astype(bf16)  # [D, S]
        shared[f"xq{b}_p"] = np.ascontiguousarray(
            xq_t.reshape(NKC, P, NQB, QB).transpose(1, 2, 0, 3))
        xk_t = kn[b].T[:, :sk].astype(bf16)
        shared[f"xk{b}_p"] = np.ascontiguousarray(
            xk_t.reshape(NKC, P, nkt[b], P).transpose(1, 2, 0, 3))
        xv_t = vb.T[:, :sk].astype(bf16)
        shared[f"xv{b}_p"] = np.ascontiguousarray(
            xv_t.reshape(NKC, P, nkt[b], P).transpose(1, 2, 0, 3))
        mt = vmask[:sk].reshape(nkt[b], P).T.astype(bf16)  # [P, nkt]
        shared[f"mask{b}_t"] = np.ascontiguousarray(
            np.repeat(mt[:, :, None], 2, axis=2))

    in_maps = []
    for core in range(8):
        gsl = slice(core * GP, (core + 1) * GP)
        m = dict(shared)
        m["wq"] = np.ascontiguousarray(wqn[:, gsl])
        m["wk"] = np.ascontiguousarray(wkn[:, gsl])
        m["wv"] = np.ascontiguousarray(wvn[:, gsl])
        m["wo"] = np.ascontiguousarray(won[gsl, :])
        in_maps.append(m)
    return in_maps, nkt


def kernel(queries, keys, values, Wq, Wk, Wv, Wo, valid_seq_lens):
    from concourse.bass_utils import run_bass_kernel_spmd

    in_maps, nkt = _prep_inputs(
        queries, keys, values, Wq, Wk, Wv, Wo, valid_seq_lens)
    nc = _build(nkt)
    res = run_bass_kernel_spmd(nc, in_maps, list(range(8)))
    out = np.zeros((B, S, D), dtype=np.float32)
    for core in range(8):
        for b in range(B):
            out[b] += res.results[core][f"out{b}"].astype(np.float32)
    return out

